# revision 61
# baseline (speedup 1.0000x reference)
"""Trainium2 Bass kernel v3: FAVOR (Performer) causal linear attention block.

Per batch element (data-parallel over 8 NeuronCores):
  c = x @ w_inp + b_inp; q,k,v = split(c)
  qf/kf = rfm_softmax(q/k, omega)             (FAVOR random feature maps)
  a     = causal_linear_attention(qf, kf, v)  (prefix outer-products + masked
                                               diagonal blocks)
  out   = a @ w_out + b_out

v3 design notes:
  - x transposed on host; QKV-feature and V GEMMs run as fp8e4 DoubleRow
    matmuls (2 k-planes per instruction, 0.5 cyc/row); weights pre-scaled
    by 64 into fp8 normal range, un-scaled via exp(s/64) activation scale
    (features) and wo/64 host fold (v path: v'=64v carried through).
  - q-side normalizer exp(-diag-m)/sqrt(F) cancels in a/denom: qf = exp(s_q).
  - k-side max taken as r = rowmax(exp(s_k)) on the bf16 feature tile;
    per-head factor applied as one broadcast DVE multiply.
  - K1 (cumulative kf sums) accumulated in a persistent PSUM pair via
    triu/strict-tril masks: 2 matmuls per block after the first.
  - attention: per-block diag scores (masked on DVE/Pool) + prefix NP
    outer-product matmuls; aT feeds output projection directly as lhsT.
"""

import numpy as np
from contextlib import ExitStack

import concourse.bass as bass
import concourse.tile as tile
from concourse import mybir
from concourse import bass_utils
import bass_rust

F32 = mybir.dt.float32
F32R = mybir.dt.float32r
BF16 = mybir.dt.bfloat16
F8 = mybir.dt.float8e4
AF = mybir.ActivationFunctionType
ALU = mybir.AluOpType
DR = mybir.MatmulPerfMode.DoubleRow

B, L, E, H, Dh, F = 8, 512, 768, 12, 64, 64
LT = L // 128       # 4 l-chunks
ET = E // 128       # 6 e-chunks
PR = ET // 2        # 3 e-pair chunks (DoubleRow planes)
NH2 = H // 2        # 6 head pairs
EPS = 1e-6
W8SCALE = 64.0
IS = 1.0 / W8SCALE

PHASES = []         # (name, first_instruction_number) markers for profiling


def _fix_waits(nc, cap=1):
    """Walrus codegen allows a single sync-wait per instruction; hoist excess
    waits onto injected same-engine NoOps placed directly before the offender
    (no reordering, deadlock-free)."""
    n = 0
    for fn in nc.m.functions:
        for bb in fn.blocks:
            insts = bb.instructions
            i = 0
            while i < len(insts):
                inst = insts[i]
                si = inst.sync_info
                if si is not None:
                    ow = list(si.on_wait)
                    if len(ow) > cap:
                        excess, keep = ow[:-cap], ow[-cap:]
                        si.on_wait = keep
                        for w in excess:
                            n += 1
                            nop = bass_rust.InstNoOp(
                                name=f"waitnop_{n}",
                                engine=inst.engine,
                                sync_info=bass_rust.SyncInfo(
                                    on_wait=[w], on_update=[]),
                            )
                            insts.insert(i, nop)
                            i += 1
                i += 1
    return n


class _Cut(Exception):
    pass


def build_nc(fix_waits=True, zb=True, cut=99):
    nc = bass.Bass("TRN2", target_bir_lowering=False, debug=False,
                   num_devices=8)
    PHASES.clear()

    def mark(name):
        PHASES.append((name, int(nc.get_next_instruction_name()[2:])))

    x8_d = nc.dram_tensor("x8", [128, PR * 2 * L], F8,
                          kind="ExternalInput").ap()
    xb_d = nc.dram_tensor("xb", [128, ET * L], BF16,
                          kind="ExternalInput").ap()
    wqk8_d = nc.dram_tensor("wqk8", [128, PR * 2 * 1536], F8,
                            kind="ExternalInput").ap()
    wvb_d = nc.dram_tensor("wvb", [128, ET * 768], BF16,
                           kind="ExternalInput").ap()
    wv8_d = nc.dram_tensor("wv8", [128, PR * 2 * 768], F8,
                           kind="ExternalInput").ap()
    wo_d = nc.dram_tensor("wo", [128, ET * 768], BF16,
                          kind="ExternalInput").ap()
    consts_d = nc.dram_tensor("consts", [128, 896 + ET * 16], BF16,
                              kind="ExternalInput").ap()
    if not zb:
        ones1_d = nc.dram_tensor("ones1", [1, 128], F32R,
                                 kind="ExternalInput").ap()
        bs_d = nc.dram_tensor("bs_rows", [2, 1536 + 16], F32R,
                              kind="ExternalInput").ap()
        b_vv_d = nc.dram_tensor("b_vv", [128, E], F32,
                                kind="ExternalInput").ap()
        b_orow_d = nc.dram_tensor("b_orow", [1, E], F32R,
                                  kind="ExternalInput").ap()
    out_d = nc.dram_tensor("out", [L, E], F32, kind="ExternalOutput").ap()

    with tile.TileContext(nc) as tc, ExitStack() as ctx:
      try:
        P = ctx.enter_context(tc.tile_pool(name="persist", bufs=1))
        st_p = ctx.enter_context(tc.tile_pool(name="stp", bufs=6))
        sm_p = ctx.enter_context(tc.tile_pool(name="smp", bufs=8))
        dn_p = ctx.enter_context(tc.tile_pool(name="dnp", bufs=2))
        osb_p = ctx.enter_context(tc.tile_pool(name="osb", bufs=2))
        ps = ctx.enter_context(tc.tile_pool(name="ps", bufs=1, space="PSUM"))

        cnt = [0]

        def pst(shape, dtype=F32, tag="big", bufs=3):
            cnt[0] += 1
            return ps.tile(shape, dtype, tag=tag, bufs=bufs,
                           name=f"pst{cnt[0]}")

        def psts(shape, dtype=F32):
            return pst(shape, dtype, tag="small", bufs=3)

        # PSUM budget: tag big x3 + small x3 + acc x2 = 8 banks.

        # Act-table warmup: absorb the 1.3us activation table load at t=0
        warm = P.tile([128, 1], F32, tag="warm", name="warm")
        nc.gpsimd.memset(warm, 0.0)
        nc.scalar.activation(warm, warm, AF.Exp)

        # ---------------- DMAs ----------------
        # SP queue spine, in critical-path order: x8, k-side weights, bf16 x
        # (pd), q-side weights. Strided q/k-half DMAs keep transfers minimal.
        x8 = P.tile([128, PR * 2 * L], F8, tag="x8", name="x8")
        x8v = x8.rearrange("p (pr two l) -> p pr two l", two=2, l=L)
        wqk8 = P.tile([128, PR * 2 * 1536], F8, tag="wqk8", name="wqk8")
        wqk8v = wqk8.rearrange("p (pr two c) -> p pr two c", two=2, c=1536)
        wqk8dv = wqk8_d.rearrange("p (pr two c) -> p pr two c", two=2, c=1536)
        xb = P.tile([128, ET * L], BF16, tag="xb", name="xb")
        xbv = xb.rearrange("p (et l) -> p et l", l=L)
        nc.sync.dma_start(out=x8, in_=x8_d)
        nc.sync.dma_start(out=wqk8v[:, :, :, 768:1536],
                          in_=wqk8dv[:, :, :, 768:1536])
        nc.sync.dma_start(out=xb, in_=xb_d)
        nc.sync.dma_start(out=wqk8v[:, :, :, 0:768],
                          in_=wqk8dv[:, :, :, 0:768])
        if not zb:
            ones1 = P.tile([1, 128], F32R, tag="ones1", name="ones1")
            nc.sync.dma_start(out=ones1, in_=ones1_d)
            bs_rows = P.tile([2, 1536 + 16], F32R, tag="bs_rows",
                             name="bs_rows")
            nc.sync.dma_start(out=bs_rows, in_=bs_d)

        # Pool (SWDGE) queue: few big DMAs (SWDGE prep ~1us each serializes
        # the queue) in need order: masks+wsum, wvb, wo, wv8.
        consts = P.tile([128, 896 + ET * 16], BF16, tag="consts",
                        name="consts")
        nc.gpsimd.dma_start(out=consts, in_=consts_d)
        maskd = consts[:, 0:128]
        maskl = consts[:, 128:256]
        maskf4 = consts[:, 256:768]
        wsumb = consts[:, 768:768 + ET * 16]
        idb = consts[:, 768 + ET * 16:896 + ET * 16]
        wvb = P.tile([128, ET * 768], BF16, tag="wvb", name="wvb")
        nc.gpsimd.dma_start(out=wvb, in_=wvb_d)
        wv8 = P.tile([128, PR * 2 * 768], F8, tag="wv8", name="wv8")
        wv8v = wv8.rearrange("p (pr two c) -> p pr two c", two=2, c=768)
        nc.gpsimd.dma_start(out=wv8, in_=wv8_d)
        wo = P.tile([128, ET * 768], BF16, tag="wo", name="wo")
        nc.gpsimd.dma_start(out=wo, in_=wo_d)
        if not zb:
            b_vv = P.tile([128, E], F32, tag="b_vv", name="b_vv")
            nc.gpsimd.dma_start(out=b_vv, in_=b_vv_d)
            b_orow = P.tile([1, E], F32R, tag="b_orow", name="b_orow")
            nc.gpsimd.dma_start(out=b_orow, in_=b_orow_d)

        # ---------------- persistent SBUF tiles ----------------
        kf = [P.tile([128, H * F], BF16, tag=f"kf{lt}", name=f"kf{lt}")
              for lt in range(LT)]
        qf = [P.tile([128, H * F], BF16, tag=f"qf{lt}", name=f"qf{lt}")
              for lt in range(LT)]
        qf_b = [P.tile([128, H * F], BF16, tag=f"qfb{lt}", name=f"qfb{lt}")
                for lt in range(LT)]
        v_p = [P.tile([128, E], BF16, tag=f"vp{lt}", name=f"vp{lt}")
               for lt in range(LT)]
        kfT_all = P.tile([128, NH2 * L], BF16, tag="kfT", name="kfT")
        kfT = [kfT_all[:, t * L:(t + 1) * L] for t in range(NH2)]
        qfT_all = P.tile([128, NH2 * L], BF16, tag="qfT", name="qfT")
        qfT = [qfT_all[:, t * L:(t + 1) * L] for t in range(NH2)]
        aTbig = P.tile([128, NH2 * L], BF16, tag="aT", name="aT")
        aT_all = [aTbig[:, t * L:(t + 1) * L] for t in range(NH2)]
        aTv = aTbig.rearrange("p (t l) -> p t l", l=L)
        # NP prefix outer products: NPs[j] = sum_{j'<=j} kf_j'^T v'_j',
        # laid out [128 (hh*64+f), NH2*F (t,d)]
        pnb = [P.tile([128, NH2 * F], BF16, tag=f"pnb{j}", name=f"pnb{j}")
               for j in range(LT - 1)]
        NPs = [P.tile([128, NH2 * F], BF16, tag=f"NP{j}", name=f"NP{j}")
               for j in range(1, LT - 1)]
        NP = [pnb[0]] + NPs  # NP[j] = prefix through block j

        # persistent K1 accumulator (2 banks)
        ka = ps.tile([128, 512], F32, tag="acc", bufs=2, name="ka")
        kb = ps.tile([128, 256], F32, tag="acc", bufs=2, name="kb")

        # ---------------- feature stage ----------------
        def qkv_mm(qk, lt, with_pd=False):
            """s[l, cols] = x @ Ws via fp8 DoubleRow; returns (sA, sB, _)."""
            sA = pst([128, 512])
            sB = pst([128, 256])
            c0 = qk * 768
            if not zb:
                nc.tensor.matmul(sA, ones1, bs_rows[qk:qk + 1, 0:512],
                                 start=True, stop=False,
                                 skip_group_check=True)
                nc.tensor.matmul(sB, ones1, bs_rows[qk:qk + 1, 512:768],
                                 start=True, stop=False,
                                 skip_group_check=True)
            for p in range(PR):
                st0 = (p == 0) and zb
                sp = (p == PR - 1)
                lhs = x8v[:, p, :, lt * 128:(lt + 1) * 128]
                nc.tensor.matmul(sA, lhs, wqk8v[:, p, :, c0:c0 + 512],
                                 start=st0, stop=sp, perf_mode=DR,
                                 skip_group_check=True)
                nc.tensor.matmul(sB, lhs, wqk8v[:, p, :, c0 + 512:c0 + 768],
                                 start=st0, stop=sp, perf_mode=DR,
                                 skip_group_check=True)
            return sA, sB, None

        def kstage_mm(lt):
            """fp8 feature matmuls + exp; pd deferred (waits on the slower
            bf16 x load) so it doesn't block the PE queue."""
            sA, sB, _ = qkv_mm(1, lt, False)
            dst = kf[lt]
            # kf_raw = exp(s) (scale 1/64 un-does the fp8 weight scaling)
            nc.scalar.activation(dst[:, 0:512], sA, AF.Exp, scale=IS)
            nc.scalar.activation(dst[:, 512:768], sB, AF.Exp, scale=IS)

        def kstage_fac(lt):
            dst = kf[lt]
            pd = psts([128, 16])
            if not zb:
                nc.tensor.matmul(pd, ones1, bs_rows[1:2, 1536:1552],
                                 start=True, stop=False,
                                 skip_group_check=True)
            for et in range(ET):
                nc.tensor.matmul(pd, xbv[:, et, lt * 128:(lt + 1) * 128],
                                 wsumb[:, et * 16:(et + 1) * 16],
                                 start=(et == 0) and zb, stop=(et == ET - 1),
                                 skip_group_check=True)
            # r = rowmax(kf_raw) = exp(m);  fac = exp(-diag)/r
            # odd blocks run the scale multiply on Pool to unload DVE
            heavy = nc.vector if lt % 2 == 0 else nc.gpsimd
            r = sm_p.tile([128, 1], F32, tag="r", name="r")
            nc.vector.reduce_max(r, dst, axis=mybir.AxisListType.X)
            fac = sm_p.tile([128, 12], F32, tag="fac", name="fac")
            # diag = 0.5 * pd (pd unscaled bf16 path)  ->  exp(-pd/2)
            nc.scalar.activation(fac, pd[:, 0:12], AF.Exp, scale=-0.5)
            rr = sm_p.tile([128, 1], F32, tag="rr", name="rr")
            with nc.allow_low_precision(reason="recip of exp(max), O(1)"):
                nc.vector.reciprocal(rr, r)
            facb = sm_p.tile([128, 12], BF16, tag="facb", name="facb")
            nc.vector.tensor_mul(facb, fac, rr.to_broadcast((128, 12)))
            # kf = kf_raw * fac (per head broadcast)
            heavy.tensor_mul(
                dst.rearrange("p (h f) -> p h f", f=F),
                dst.rearrange("p (h f) -> p h f", f=F),
                facb.to_broadcast((128, 12, F)))
            for t in range(NH2):
                nc.sync.dma_start(
                    out=kfT[t][:, lt * 128:(lt + 1) * 128],
                    in_=dst[:, t * 128:(t + 1) * 128], transpose=True)

        def vstage(lt):
            """v' = 64*v. Block 0 runs bf16 (low-support early positions
            see v errors unaveraged); later blocks run fp8 DoubleRow."""
            pv1 = pst([128, 512])
            pv2 = pst([128, 256])
            if lt == 0:
                for et in range(ET):
                    st0 = et == 0
                    sp = et == ET - 1
                    lhs = xbv[:, et, lt * 128:(lt + 1) * 128]
                    nc.tensor.matmul(pv1, lhs,
                                     wvb[:, et * 768:et * 768 + 512],
                                     start=st0, stop=sp,
                                     skip_group_check=True)
                    nc.tensor.matmul(pv2, lhs,
                                     wvb[:, et * 768 + 512:(et + 1) * 768],
                                     start=st0, stop=sp,
                                     skip_group_check=True)
            else:
                for p in range(PR):
                    st0 = p == 0
                    sp = p == PR - 1
                    lhs = x8v[:, p, :, lt * 128:(lt + 1) * 128]
                    nc.tensor.matmul(pv1, lhs, wv8v[:, p, :, 0:512],
                                     start=st0, stop=sp, perf_mode=DR,
                                     skip_group_check=True)
                    nc.tensor.matmul(pv2, lhs, wv8v[:, p, :, 512:768],
                                     start=st0, stop=sp, perf_mode=DR,
                                     skip_group_check=True)
            # v' = 64*v kept scaled; un-scaled via wo/64 host fold
            if zb:
                nc.scalar.copy(v_p[lt][:, 0:512], pv1)
                nc.scalar.copy(v_p[lt][:, 512:768], pv2)
            else:
                # v' = psum + 64*b_v  (b_vv host-prescaled by 64)
                nc.vector.tensor_add(v_p[lt][:, 0:512], pv1, b_vv[:, 0:512])
                nc.vector.tensor_add(v_p[lt][:, 512:768], pv2,
                                     b_vv[:, 512:768])

        def njstage(lt):
            # N_lt[f, (t,d)] = kf_lt^T v'_lt per head, hh packed on partitions
            pn = pst([128, NH2 * F])
            for t in range(NH2):
                for hh in range(2):
                    h = 2 * t + hh
                    nc.tensor.matmul(
                        pn[hh * 64:hh * 64 + 64, t * F:(t + 1) * F],
                        kf[lt][:, h * F:(h + 1) * F],
                        v_p[lt][:, h * F:(h + 1) * F],
                        start=True, stop=True, skip_group_check=True)
            nc.scalar.copy(pnb[lt], pn)

        # ---------------- q stage (features + denominator) ----------------
        def qstage_mm(i):
            return qkv_mm(0, i, False)

        def qstage_exp(i, sA, sB):
            nc.scalar.activation(qf[i][:, 0:512], sA, AF.Exp, scale=IS)
            nc.scalar.activation(qf[i][:, 512:768], sB, AF.Exp, scale=IS)

        def k1stage(i):
            # ka/kb accumulate K1 for block i: add strict-lower of block i-1
            # (completing its full sum), then masked-diag of block i.
            if i > 0:
                nc.tensor.matmul(ka, maskl, kf[i - 1][:, 0:512],
                                 start=False, stop=False,
                                 skip_group_check=True)
                nc.tensor.matmul(kb, maskl, kf[i - 1][:, 512:768],
                                 start=False, stop=False,
                                 skip_group_check=True)
            nc.tensor.matmul(ka, maskd, kf[i][:, 0:512],
                             start=(i == 0), stop=(i == LT - 1),
                             skip_group_check=True)
            nc.tensor.matmul(kb, maskd, kf[i][:, 512:768],
                             start=(i == 0), stop=(i == LT - 1),
                             skip_group_check=True)

        def denstage(i):
            # den = qf . K1 per head; rq = 1/den (EPS dropped: den >= ~3e-3)
            dn = dn_p.tile([128, H * F], BF16, tag="dn", name="dn")
            nc.vector.tensor_mul(dn[:, 0:512], qf[i][:, 0:512], ka)
            nc.vector.tensor_mul(dn[:, 512:768], qf[i][:, 512:768], kb)
            den = sm_p.tile([128, 12], F32, tag="den", name="den")
            nc.vector.reduce_sum(den, dn.rearrange("p (h f) -> p h f", f=F),
                                 axis=mybir.AxisListType.X)
            rqb = sm_p.tile([128, 12], BF16, tag="rqb", name="rqb")
            with nc.allow_low_precision(reason="recip of O(100) denom"):
                nc.vector.reciprocal(rqb, den)
            nc.vector.tensor_mul(
                qf_b[i].rearrange("p (h f) -> p h f", f=F),
                qf[i].rearrange("p (h f) -> p h f", f=F),
                rqb.to_broadcast((128, 12, F)))

        def qtstage(i):
            for t in range(NH2):
                nc.sync.dma_start(
                    out=qfT[t][:, i * 128:(i + 1) * 128],
                    in_=qf_b[i][:, t * 128:(t + 1) * 128], transpose=True)

        # ---------------- attention + output projection ----------------
        def scores_t(i, t, on_dve):
            """Diag-block scores for head pair t: two 64-contraction matmuls
            into per-hh [128,128] psum tiles (baseline-proven shapes)."""
            sts = []
            for hh in range(2):
                pq = psts([128, 128])
                nc.tensor.matmul(
                    pq,
                    kfT[t][hh * 64:hh * 64 + 64, i * 128:(i + 1) * 128],
                    qfT[t][hh * 64:hh * 64 + 64, i * 128:(i + 1) * 128],
                    start=True, stop=True)
                st = st_p.tile([128, 128], BF16, tag="st", name="st")
                if on_dve:
                    nc.vector.tensor_mul(st, pq, maskf4[:, 0:128])
                else:
                    nc.scalar.copy(st, pq)
                    nc.gpsimd.tensor_mul(st, st, maskf4[:, 0:128])
                sts.append(st)
            return sts

        def pa_t(i, t, sts, paqA, paqB):
            """Attention for head pair t into the packed psum (baseline
            layout: t 0-3 in paqA columns, t 4-5 in paqB)."""
            pa = (paqA[:, (t % 4) * 128:(t % 4) * 128 + 128] if t < 4
                  else paqB[:, (t - 4) * 128:(t - 4) * 128 + 128])
            for hh in range(2):
                h = 2 * t + hh
                dst = pa[hh * 64:hh * 64 + 64, :]
                if i > 0:
                    nc.tensor.matmul(
                        dst,
                        NP[i - 1][hh * 64:hh * 64 + 64, t * F:(t + 1) * F],
                        qfT[t][hh * 64:hh * 64 + 64, i * 128:(i + 1) * 128],
                        start=True, stop=False, skip_group_check=True)
                nc.tensor.matmul(
                    dst, v_p[i][:, h * F:(h + 1) * F], sts[hh],
                    start=(i == 0), stop=True, skip_group_check=True)

        def aT_evac_pair(i, t, paqA, paqB, on_dve):
            """Evacuate heads pairs t-1, t (t odd) like the baseline."""
            if t < 4:
                src = (paqA.rearrange("p (t l) -> p t l", l=128)
                       [:, t - 1:t + 1, :])
            else:
                src = paqB.rearrange("p (t l) -> p t l", l=128)
            dst = aTv[:, t - 1:t + 1, i * 128:(i + 1) * 128]
            if on_dve:
                nc.vector.tensor_copy(dst, src)
            else:
                nc.scalar.copy(dst, src)

        def outproj(i, tt, po1, po2):
            st0 = zb and tt == 0
            sp = tt == NH2 - 1
            nc.tensor.matmul(po1, aT_all[tt][:, i * 128:(i + 1) * 128],
                             wo[:, tt * 768:tt * 768 + 512],
                             start=st0, stop=sp, skip_group_check=True)
            nc.tensor.matmul(po2, aT_all[tt][:, i * 128:(i + 1) * 128],
                             wo[:, tt * 768 + 512:tt * 768 + 768],
                             start=st0, stop=sp, skip_group_check=True)

        def iblk(i, extra=()):
            """Attention + output projection for block i; `extra` stages are
            interleaved to fill engine slack."""
            extra = list(extra)
            po1 = pst([128, 512])
            po2 = pst([128, 256], tag="small", bufs=3)
            if not zb:
                nc.tensor.matmul(po1, ones1, b_orow[0:1, 0:512],
                                 start=True, stop=False,
                                 skip_group_check=True)
                nc.tensor.matmul(po2, ones1, b_orow[0:1, 512:768],
                                 start=True, stop=False,
                                 skip_group_check=True)
            paqA = pst([128, 512])
            paqB = pst([128, 256])
            for t in range(NH2):
                sts = scores_t(i, t, t % 2 == 0)
                if t % 2 == 1 and extra:
                    extra.pop(0)()
                pa_t(i, t, sts, paqA, paqB)
                if t % 2 == 1:
                    aT_evac_pair(i, t, paqA, paqB, t == 3)
                    outproj(i, t - 1, po1, po2)
                    outproj(i, t, po1, po2)
            # output: evac + DMA (DVE/Act split + chunked DMAs to minimize
            # the kernel tail)
            osb = osb_p.tile([128, E], F32, tag="osb", name="osb")
            if i == LT - 1:
                nc.vector.tensor_copy(osb[:, 0:256], po1[:, 0:256])
                nc.scalar.copy(osb[:, 512:768], po2)
                nc.vector.tensor_copy(osb[:, 256:512], po1[:, 256:512])
                for c0 in (512, 0, 256):
                    nc.sync.dma_start(
                        out=out_d[i * 128:(i + 1) * 128, c0:c0 + 256],
                        in_=osb[:, c0:c0 + 256])
            else:
                nc.vector.tensor_copy(osb[:, 0:512], po1)
                nc.scalar.copy(osb[:, 512:768], po2)
                nc.sync.dma_start(
                    out=out_d[i * 128:(i + 1) * 128, 0:512],
                    in_=osb[:, 0:512])
                nc.sync.dma_start(
                    out=out_d[i * 128:(i + 1) * 128, 512:768],
                    in_=osb[:, 512:768])
            for fn in extra:
                fn()

        def qstage(j):
            sA, sB, _ = qkv_mm(0, j)
            qstage_exp(j, sA, sB)

        def bstage(j):
            k1stage(j)
            denstage(j)
            qtstage(j)

        # ================= emission =================
        def ckpt(n):
            if cut <= n or n == -1:
                # dummy out writes so the output is retrievable
                dummy = P.tile([128, E], F32, tag="dummy", name="dummy")
                nc.gpsimd.memset(dummy, 0.0)
                for ii in range(LT):
                    nc.sync.dma_start(
                        out=out_d[ii * 128:(ii + 1) * 128, :], in_=dummy)
                raise _Cut

        # Priority spine: k features -> q features -> K1/den/qfT chains,
        # with v projections and attention blocks filling in behind.
        ckpt(0)
        mark("kstage0")
        kstage_mm(0)
        mark("qstage0")
        qstage(0)
        ckpt(1)
        mark("kfac0")
        kstage_fac(0)
        ckpt(2)
        mark("bstage0")
        bstage(0)
        mark("kmm1")
        kstage_mm(1)
        kstage_fac(1)
        mark("qstage1")
        qstage(1)
        mark("bstage1")
        bstage(1)
        ckpt(3)
        mark("kmm23")
        kstage_mm(2)
        kstage_fac(2)
        kstage_mm(3)
        kstage_fac(3)
        ckpt(4)
        mark("vstage0")
        vstage(0)
        njstage(0)
        mark("qstage2")
        qstage(2)
        mark("vstage1")
        vstage(1)
        njstage(1)
        nc.gpsimd.tensor_add(NP[1], NP[0], pnb[1])
        mark("vstage23")
        vstage(2)
        njstage(2)
        nc.gpsimd.tensor_add(NP[2], NP[1], pnb[2])
        vstage(3)

        ckpt(5)
        mark("iblk0")
        iblk(0, extra=[lambda: (k1stage(2), denstage(2)),
                       lambda: qtstage(2)])
        mark("qstage3")
        qstage(3)
        mark("iblk1")
        iblk(1, extra=[lambda: (k1stage(3), denstage(3)),
                       lambda: qtstage(3)])
        ckpt(6)
        mark("iblk2")
        iblk(2)
        mark("iblk3")
        iblk(3)

      except _Cut:
        pass

    if fix_waits:
        # production wait-splitting: matmul waits migrate to ldweights, the
        # rest are split via InstEventSemaphore (walrus allows 1 wait/inst)
        bass_rust.move_matmul_waits_to_ldweights(nc.m)
        bass_rust.generate_event_semaphores(nc)
    return nc


_CACHE = {}


def _host_consts(wsumb):
    import ml_dtypes
    bf = ml_dtypes.bfloat16
    tri = np.triu(np.ones((128, 128), dtype=np.float32))
    masks = np.concatenate(
        [tri, np.tril(np.ones((128, 128), dtype=np.float32), -1),
         np.tile(tri, (1, 4))], axis=1).astype(bf)
    idb = np.eye(128, dtype=np.float32).astype(bf)
    return {"consts": np.concatenate([masks, wsumb, idb], axis=1)}


def _pair_pack(w, cols):
    """[768, cols] -> [128, PR*2*cols] fp8 e-pair/plane-major layout."""
    import ml_dtypes
    f8 = ml_dtypes.float8_e4m3
    return np.ascontiguousarray(
        w.reshape(PR, 2, 128, cols).transpose(2, 0, 1, 3)
        .reshape(128, PR * 2 * cols)).astype(f8)


def _in_maps(x, w_inp, b_inp, w_out, b_out, omega):
    import ml_dtypes
    bf = ml_dtypes.bfloat16
    f = lambda a: np.ascontiguousarray(np.asarray(a), dtype=np.float32)
    x, w_inp, b_inp = f(x), f(w_inp), f(b_inp)
    w_out, b_out, omega = f(w_out), f(b_out), f(omega)
    w = w_inp[0]  # [E, 3E]
    omt = (omega.T * (float(Dh) ** -0.25)).astype(np.float64)   # [d, f]
    # fold omega into the q/k projections: Ws[:, (qk,h,f)] per head
    ws = np.empty((E, 1536), np.float64)
    wqk_full = w[:, 0:1536].astype(np.float64)
    for qk in range(2):
        for h in range(H):
            c = qk * 768 + h * 64
            ws[:, c:c + 64] = wqk_full[:, c:c + 64] @ omt
    # k-side per-head column sums (diag), padded 12->16, bf16 et-major
    wsum_full = np.zeros((E, 16), np.float64)
    wsum_full[:, 0:12] = ws[:, 768:1536].reshape(E, 12, 64).sum(axis=2)
    wqk8 = _pair_pack((ws * W8SCALE).astype(np.float32), 1536)
    wsumb = np.ascontiguousarray(
        wsum_full.astype(np.float32).reshape(ET, 128, 16)
        .transpose(1, 0, 2).reshape(128, ET * 16)).astype(bf)
    consts = _host_consts(wsumb)
    wv8 = _pair_pack(w[:, 1536:2304] * W8SCALE, 768)
    # bf16 v weights (block 0), same x64 scale so v'=64v uniformly
    wvb = np.ascontiguousarray(
        (w[:, 1536:2304] * W8SCALE).reshape(ET, 128, 768)
        .transpose(1, 0, 2).reshape(128, ET * 768)).astype(bf)
    # wo/64 un-does the v'=64v scaling
    wo = np.ascontiguousarray(
        (w_out[0] * IS).reshape(ET, 128, 768).transpose(1, 0, 2)
        .reshape(128, ET * 768)).astype(bf)
    zb = bool(np.all(b_inp == 0.0) and np.all(b_out == 0.0))
    maps = []
    for c in range(B):
        xT = x[c].T
        x8 = _pair_pack(xT, L)
        xbn = np.ascontiguousarray(
            xT.reshape(ET, 128, L).transpose(1, 0, 2)
            .reshape(128, ET * L)).astype(bf)
        m = {"x8": x8, "xb": xbn, "wqk8": wqk8, "wv8": wv8, "wvb": wvb,
             "wo": wo}
        if not zb:
            bs = np.zeros((2, 1536 + 16), np.float32)
            for qk in range(2):
                bq = b_inp[qk * 768:(qk + 1) * 768].astype(np.float64)
                bsh = np.empty((768,), np.float64)
                for h in range(H):
                    bsh[h * 64:(h + 1) * 64] = bq[h * 64:(h + 1) * 64] @ omt
                # bias rows feed the x64-scaled psum: multiply by 64;
                # the pd psum is unscaled bf16: sums stay unscaled
                bs[qk, 0:768] = (bsh * W8SCALE).astype(np.float32)
                if qk == 1:
                    bs[1, 1536:1548] = (
                        bsh.reshape(12, 64).sum(axis=1)).astype(np.float32)
            m["bs_rows"] = bs
            m["ones1"] = np.ones((1, 128), np.float32)
            m["b_vv"] = np.ascontiguousarray(np.broadcast_to(
                b_inp[1536:2304] * W8SCALE, (128, E))).astype(np.float32)
            m["b_orow"] = np.ascontiguousarray(b_out).reshape(1, E)
        m.update(consts)
        maps.append(m)
    return maps


def kernel(x, w_inp, b_inp, w_out, b_out, omega):
    maps = _in_maps(x, w_inp, b_inp, w_out, b_out, omega)
    zb = "b_vv" not in maps[0]
    key = f"nc{int(zb)}"
    if key not in _CACHE:
        _CACHE[key] = build_nc(zb=zb)
    nc = _CACHE[key]
    res = bass_utils.run_bass_kernel_spmd(nc, maps, core_ids=list(range(B)))
    return np.stack([res.results[c]["out"] for c in range(B)])


# revision 62
# speedup vs baseline: 1.0456x; 1.0456x over previous
"""Trainium2 Bass kernel v3: FAVOR (Performer) causal linear attention block.

Per batch element (data-parallel over 8 NeuronCores):
  c = x @ w_inp + b_inp; q,k,v = split(c)
  qf/kf = rfm_softmax(q/k, omega)             (FAVOR random feature maps)
  a     = causal_linear_attention(qf, kf, v)  (prefix outer-products + masked
                                               diagonal blocks)
  out   = a @ w_out + b_out

v3 design notes:
  - x transposed on host; QKV-feature and V GEMMs run as fp8e4 DoubleRow
    matmuls (2 k-planes per instruction, 0.5 cyc/row); weights pre-scaled
    by 64 into fp8 normal range, un-scaled via exp(s/64) activation scale
    (features) and wo/64 host fold (v path: v'=64v carried through).
  - q-side normalizer exp(-diag-m)/sqrt(F) cancels in a/denom: qf = exp(s_q).
  - k-side max taken as r = rowmax(exp(s_k)) on the bf16 feature tile;
    per-head factor applied as one broadcast DVE multiply.
  - K1 (cumulative kf sums) accumulated in a persistent PSUM pair via
    triu/strict-tril masks: 2 matmuls per block after the first.
  - attention: per-block diag scores (masked on DVE/Pool) + prefix NP
    outer-product matmuls; aT feeds output projection directly as lhsT.
"""

import numpy as np
from contextlib import ExitStack

import concourse.bass as bass
import concourse.tile as tile
from concourse import mybir
from concourse import bass_utils
import bass_rust

F32 = mybir.dt.float32
F32R = mybir.dt.float32r
BF16 = mybir.dt.bfloat16
F8 = mybir.dt.float8e4
AF = mybir.ActivationFunctionType
ALU = mybir.AluOpType
DR = mybir.MatmulPerfMode.DoubleRow

B, L, E, H, Dh, F = 8, 512, 768, 12, 64, 64
LT = L // 128       # 4 l-chunks
ET = E // 128       # 6 e-chunks
PR = ET // 2        # 3 e-pair chunks (DoubleRow planes)
NH2 = H // 2        # 6 head pairs
EPS = 1e-6
W8SCALE = 64.0
IS = 1.0 / W8SCALE

PHASES = []         # (name, first_instruction_number) markers for profiling


def _fix_waits(nc, cap=1):
    """Walrus codegen allows a single sync-wait per instruction; hoist excess
    waits onto injected same-engine NoOps placed directly before the offender
    (no reordering, deadlock-free)."""
    n = 0
    for fn in nc.m.functions:
        for bb in fn.blocks:
            insts = bb.instructions
            i = 0
            while i < len(insts):
                inst = insts[i]
                si = inst.sync_info
                if si is not None:
                    ow = list(si.on_wait)
                    if len(ow) > cap:
                        excess, keep = ow[:-cap], ow[-cap:]
                        si.on_wait = keep
                        for w in excess:
                            n += 1
                            nop = bass_rust.InstNoOp(
                                name=f"waitnop_{n}",
                                engine=inst.engine,
                                sync_info=bass_rust.SyncInfo(
                                    on_wait=[w], on_update=[]),
                            )
                            insts.insert(i, nop)
                            i += 1
                i += 1
    return n


class _Cut(Exception):
    pass


def build_nc(fix_waits=True, zb=True, cut=99):
    nc = bass.Bass("TRN2", target_bir_lowering=False, debug=False,
                   num_devices=8)
    PHASES.clear()

    def mark(name):
        PHASES.append((name, int(nc.get_next_instruction_name()[2:])))

    x8_d = nc.dram_tensor("x8", [128, PR * 2 * L], F8,
                          kind="ExternalInput").ap()
    xb_d = nc.dram_tensor("xb", [128, ET * L], BF16,
                          kind="ExternalInput").ap()
    wqk8_d = nc.dram_tensor("wqk8", [128, PR * 2 * 1536], F8,
                            kind="ExternalInput").ap()
    wvb_d = nc.dram_tensor("wvb", [128, ET * 768], BF16,
                           kind="ExternalInput").ap()
    wv8_d = nc.dram_tensor("wv8", [128, PR * 2 * 768], F8,
                           kind="ExternalInput").ap()
    wo_d = nc.dram_tensor("wo", [128, ET * 768], BF16,
                          kind="ExternalInput").ap()
    consts_d = nc.dram_tensor("consts", [128, 896 + ET * 16], BF16,
                              kind="ExternalInput").ap()
    if not zb:
        ones1_d = nc.dram_tensor("ones1", [1, 128], F32R,
                                 kind="ExternalInput").ap()
        bs_d = nc.dram_tensor("bs_rows", [2, 1536 + 16], F32R,
                              kind="ExternalInput").ap()
        b_vv_d = nc.dram_tensor("b_vv", [128, E], F32,
                                kind="ExternalInput").ap()
        b_orow_d = nc.dram_tensor("b_orow", [1, E], F32R,
                                  kind="ExternalInput").ap()
    out_d = nc.dram_tensor("out", [L, E], F32, kind="ExternalOutput").ap()

    with tile.TileContext(nc) as tc, ExitStack() as ctx:
      try:
        P = ctx.enter_context(tc.tile_pool(name="persist", bufs=1))
        st_p = ctx.enter_context(tc.tile_pool(name="stp", bufs=6))
        sm_p = ctx.enter_context(tc.tile_pool(name="smp", bufs=8))
        dn_p = ctx.enter_context(tc.tile_pool(name="dnp", bufs=2))
        osb_p = ctx.enter_context(tc.tile_pool(name="osb", bufs=2))
        ps = ctx.enter_context(tc.tile_pool(name="ps", bufs=1, space="PSUM"))

        cnt = [0]

        def pst(shape, dtype=F32, tag="big", bufs=4):
            cnt[0] += 1
            return ps.tile(shape, dtype, tag=tag, bufs=bufs,
                           name=f"pst{cnt[0]}")

        def psts(shape, dtype=F32):
            return pst(shape, dtype, tag="small", bufs=2)

        # PSUM budget: tag big x3 + small x3 + acc x2 = 8 banks.

        # Act-table warmup: absorb the 1.3us activation table load at t=0
        warm = P.tile([128, 1], F32, tag="warm", name="warm")
        nc.gpsimd.memset(warm, 0.0)
        nc.scalar.activation(warm, warm, AF.Exp)

        # ---------------- DMAs ----------------
        # SP queue spine, in critical-path order: x8, k-side weights, bf16 x
        # (pd), q-side weights. Strided q/k-half DMAs keep transfers minimal.
        x8 = P.tile([128, PR * 2 * L], F8, tag="x8", name="x8")
        x8v = x8.rearrange("p (pr two l) -> p pr two l", two=2, l=L)
        wqk8 = P.tile([128, PR * 2 * 1536], F8, tag="wqk8", name="wqk8")
        wqk8v = wqk8.rearrange("p (pr two c) -> p pr two c", two=2, c=1536)
        wqk8dv = wqk8_d.rearrange("p (pr two c) -> p pr two c", two=2, c=1536)
        xb = P.tile([128, ET * L], BF16, tag="xb", name="xb")
        xbv = xb.rearrange("p (et l) -> p et l", l=L)
        nc.sync.dma_start(out=x8, in_=x8_d)
        nc.sync.dma_start(out=wqk8v[:, :, :, 768:1536],
                          in_=wqk8dv[:, :, :, 768:1536])
        nc.sync.dma_start(out=xb, in_=xb_d)
        nc.sync.dma_start(out=wqk8v[:, :, :, 0:768],
                          in_=wqk8dv[:, :, :, 0:768])
        if not zb:
            ones1 = P.tile([1, 128], F32R, tag="ones1", name="ones1")
            nc.sync.dma_start(out=ones1, in_=ones1_d)
            bs_rows = P.tile([2, 1536 + 16], F32R, tag="bs_rows",
                             name="bs_rows")
            nc.sync.dma_start(out=bs_rows, in_=bs_d)

        # Pool (SWDGE) queue: few big DMAs (SWDGE prep ~1us each serializes
        # the queue) in need order: masks+wsum, wvb, wo, wv8.
        consts = P.tile([128, 896 + ET * 16], BF16, tag="consts",
                        name="consts")
        nc.gpsimd.dma_start(out=consts, in_=consts_d)
        maskd = consts[:, 0:128]
        maskl = consts[:, 128:256]
        maskf4 = consts[:, 256:768]
        wsumb = consts[:, 768:768 + ET * 16]
        idb = consts[:, 768 + ET * 16:896 + ET * 16]
        wvb = P.tile([128, ET * 768], BF16, tag="wvb", name="wvb")
        nc.gpsimd.dma_start(out=wvb, in_=wvb_d)
        wv8 = P.tile([128, PR * 2 * 768], F8, tag="wv8", name="wv8")
        wv8v = wv8.rearrange("p (pr two c) -> p pr two c", two=2, c=768)
        nc.gpsimd.dma_start(out=wv8, in_=wv8_d)
        wo = P.tile([128, ET * 768], BF16, tag="wo", name="wo")
        nc.gpsimd.dma_start(out=wo, in_=wo_d)
        if not zb:
            b_vv = P.tile([128, E], F32, tag="b_vv", name="b_vv")
            nc.gpsimd.dma_start(out=b_vv, in_=b_vv_d)
            b_orow = P.tile([1, E], F32R, tag="b_orow", name="b_orow")
            nc.gpsimd.dma_start(out=b_orow, in_=b_orow_d)

        # ---------------- persistent SBUF tiles ----------------
        kf = [P.tile([128, H * F], BF16, tag=f"kf{lt}", name=f"kf{lt}")
              for lt in range(LT)]
        qf = [P.tile([128, H * F], BF16, tag=f"qf{lt}", name=f"qf{lt}")
              for lt in range(LT)]
        qf_b = [P.tile([128, H * F], BF16, tag=f"qfb{lt}", name=f"qfb{lt}")
                for lt in range(LT)]
        v_p = [P.tile([128, E], BF16, tag=f"vp{lt}", name=f"vp{lt}")
               for lt in range(LT)]
        kfT_all = P.tile([128, NH2 * L], BF16, tag="kfT", name="kfT")
        kfT = [kfT_all[:, t * L:(t + 1) * L] for t in range(NH2)]
        qfT_all = P.tile([128, NH2 * L], BF16, tag="qfT", name="qfT")
        qfT = [qfT_all[:, t * L:(t + 1) * L] for t in range(NH2)]
        aTbig = P.tile([128, NH2 * L], BF16, tag="aT", name="aT")
        aT_all = [aTbig[:, t * L:(t + 1) * L] for t in range(NH2)]
        aTv = aTbig.rearrange("p (t l) -> p t l", l=L)
        # NP prefix outer products: NPs[j] = sum_{j'<=j} kf_j'^T v'_j',
        # laid out [128 (hh*64+f), NH2*F (t,d)]
        pnb = [P.tile([128, NH2 * F], BF16, tag=f"pnb{j}", name=f"pnb{j}")
               for j in range(LT - 1)]
        NPs = [P.tile([128, NH2 * F], BF16, tag=f"NP{j}", name=f"NP{j}")
               for j in range(1, LT - 1)]
        NP = [pnb[0]] + NPs  # NP[j] = prefix through block j

        # persistent K1 accumulator (2 banks)
        ka = ps.tile([128, 512], F32, tag="acc", bufs=2, name="ka")
        kb = ps.tile([128, 256], F32, tag="acc", bufs=2, name="kb")

        # ---------------- feature stage ----------------
        def qkv_mm(qk, lt, with_pd=False):
            """s[l, cols] = x @ Ws via fp8 DoubleRow; returns (sA, sB, _)."""
            sA = pst([128, 512])
            sB = pst([128, 256])
            c0 = qk * 768
            if not zb:
                nc.tensor.matmul(sA, ones1, bs_rows[qk:qk + 1, 0:512],
                                 start=True, stop=False,
                                 skip_group_check=True)
                nc.tensor.matmul(sB, ones1, bs_rows[qk:qk + 1, 512:768],
                                 start=True, stop=False,
                                 skip_group_check=True)
            for p in range(PR):
                st0 = (p == 0) and zb
                sp = (p == PR - 1)
                lhs = x8v[:, p, :, lt * 128:(lt + 1) * 128]
                nc.tensor.matmul(sA, lhs, wqk8v[:, p, :, c0:c0 + 512],
                                 start=st0, stop=sp, perf_mode=DR,
                                 skip_group_check=True)
                nc.tensor.matmul(sB, lhs, wqk8v[:, p, :, c0 + 512:c0 + 768],
                                 start=st0, stop=sp, perf_mode=DR,
                                 skip_group_check=True)
            return sA, sB, None

        def kstage_mm(lt):
            """fp8 feature matmuls + exp; pd deferred (waits on the slower
            bf16 x load) so it doesn't block the PE queue."""
            sA, sB, _ = qkv_mm(1, lt, False)
            dst = kf[lt]
            # kf_raw = exp(s) (scale 1/64 un-does the fp8 weight scaling)
            nc.scalar.activation(dst[:, 0:512], sA, AF.Exp, scale=IS)
            nc.scalar.activation(dst[:, 512:768], sB, AF.Exp, scale=IS)

        def kstage_fac(lt):
            dst = kf[lt]
            pd = psts([128, 16])
            if not zb:
                nc.tensor.matmul(pd, ones1, bs_rows[1:2, 1536:1552],
                                 start=True, stop=False,
                                 skip_group_check=True)
            for et in range(ET):
                nc.tensor.matmul(pd, xbv[:, et, lt * 128:(lt + 1) * 128],
                                 wsumb[:, et * 16:(et + 1) * 16],
                                 start=(et == 0) and zb, stop=(et == ET - 1),
                                 skip_group_check=True)
            # r = rowmax(kf_raw) = exp(m);  fac = exp(-diag)/r
            # odd blocks run the scale multiply on Pool to unload DVE
            heavy = nc.vector if lt % 2 == 0 else nc.gpsimd
            r = sm_p.tile([128, 1], F32, tag="r", name="r")
            nc.vector.reduce_max(r, dst, axis=mybir.AxisListType.X)
            fac = sm_p.tile([128, 12], F32, tag="fac", name="fac")
            # diag = 0.5 * pd (pd unscaled bf16 path)  ->  exp(-pd/2)
            nc.scalar.activation(fac, pd[:, 0:12], AF.Exp, scale=-0.5)
            rr = sm_p.tile([128, 1], F32, tag="rr", name="rr")
            with nc.allow_low_precision(reason="recip of exp(max), O(1)"):
                nc.vector.reciprocal(rr, r)
            facb = sm_p.tile([128, 12], BF16, tag="facb", name="facb")
            nc.vector.tensor_mul(facb, fac, rr.to_broadcast((128, 12)))
            # kf = kf_raw * fac (per head broadcast)
            heavy.tensor_mul(
                dst.rearrange("p (h f) -> p h f", f=F),
                dst.rearrange("p (h f) -> p h f", f=F),
                facb.to_broadcast((128, 12, F)))
            for t in range(NH2):
                nc.sync.dma_start(
                    out=kfT[t][:, lt * 128:(lt + 1) * 128],
                    in_=dst[:, t * 128:(t + 1) * 128], transpose=True)

        def vstage(lt):
            """v' = 64*v. Block 0 runs bf16 (low-support early positions
            see v errors unaveraged); later blocks run fp8 DoubleRow."""
            pv1 = pst([128, 512])
            pv2 = pst([128, 256])
            if lt == 0:
                for et in range(ET):
                    st0 = et == 0
                    sp = et == ET - 1
                    lhs = xbv[:, et, lt * 128:(lt + 1) * 128]
                    nc.tensor.matmul(pv1, lhs,
                                     wvb[:, et * 768:et * 768 + 512],
                                     start=st0, stop=sp,
                                     skip_group_check=True)
                    nc.tensor.matmul(pv2, lhs,
                                     wvb[:, et * 768 + 512:(et + 1) * 768],
                                     start=st0, stop=sp,
                                     skip_group_check=True)
            else:
                for p in range(PR):
                    st0 = p == 0
                    sp = p == PR - 1
                    lhs = x8v[:, p, :, lt * 128:(lt + 1) * 128]
                    nc.tensor.matmul(pv1, lhs, wv8v[:, p, :, 0:512],
                                     start=st0, stop=sp, perf_mode=DR,
                                     skip_group_check=True)
                    nc.tensor.matmul(pv2, lhs, wv8v[:, p, :, 512:768],
                                     start=st0, stop=sp, perf_mode=DR,
                                     skip_group_check=True)
            # v' = 64*v kept scaled; un-scaled via wo/64 host fold
            if zb:
                nc.scalar.copy(v_p[lt][:, 0:512], pv1)
                nc.scalar.copy(v_p[lt][:, 512:768], pv2)
            else:
                # v' = psum + 64*b_v  (b_vv host-prescaled by 64)
                nc.vector.tensor_add(v_p[lt][:, 0:512], pv1, b_vv[:, 0:512])
                nc.vector.tensor_add(v_p[lt][:, 512:768], pv2,
                                     b_vv[:, 512:768])

        def njstage(lt):
            # N_lt[f, (t,d)] = kf_lt^T v'_lt per head, hh packed on partitions
            pn = pst([128, NH2 * F])
            for t in range(NH2):
                for hh in range(2):
                    h = 2 * t + hh
                    nc.tensor.matmul(
                        pn[hh * 64:hh * 64 + 64, t * F:(t + 1) * F],
                        kf[lt][:, h * F:(h + 1) * F],
                        v_p[lt][:, h * F:(h + 1) * F],
                        start=True, stop=True, skip_group_check=True)
            nc.scalar.copy(pnb[lt], pn)

        # ---------------- q stage (features + denominator) ----------------
        def qstage_mm(i):
            return qkv_mm(0, i, False)

        def qstage_exp(i, sA, sB):
            nc.scalar.activation(qf[i][:, 0:512], sA, AF.Exp, scale=IS)
            nc.scalar.activation(qf[i][:, 512:768], sB, AF.Exp, scale=IS)

        def k1stage(i):
            # ka/kb accumulate K1 for block i: add strict-lower of block i-1
            # (completing its full sum), then masked-diag of block i.
            if i > 0:
                nc.tensor.matmul(ka, maskl, kf[i - 1][:, 0:512],
                                 start=False, stop=False,
                                 skip_group_check=True)
                nc.tensor.matmul(kb, maskl, kf[i - 1][:, 512:768],
                                 start=False, stop=False,
                                 skip_group_check=True)
            nc.tensor.matmul(ka, maskd, kf[i][:, 0:512],
                             start=(i == 0), stop=(i == LT - 1),
                             skip_group_check=True)
            nc.tensor.matmul(kb, maskd, kf[i][:, 512:768],
                             start=(i == 0), stop=(i == LT - 1),
                             skip_group_check=True)

        def denstage(i):
            # den = qf . K1 per head; rq = 1/den (EPS dropped: den >= ~3e-3)
            dn = dn_p.tile([128, H * F], BF16, tag="dn", name="dn")
            nc.vector.tensor_mul(dn[:, 0:512], qf[i][:, 0:512], ka)
            nc.vector.tensor_mul(dn[:, 512:768], qf[i][:, 512:768], kb)
            den = sm_p.tile([128, 12], F32, tag="den", name="den")
            nc.vector.reduce_sum(den, dn.rearrange("p (h f) -> p h f", f=F),
                                 axis=mybir.AxisListType.X)
            rqb = sm_p.tile([128, 12], BF16, tag="rqb", name="rqb")
            with nc.allow_low_precision(reason="recip of O(100) denom"):
                nc.vector.reciprocal(rqb, den)
            nc.vector.tensor_mul(
                qf_b[i].rearrange("p (h f) -> p h f", f=F),
                qf[i].rearrange("p (h f) -> p h f", f=F),
                rqb.to_broadcast((128, 12, F)))

        def qtstage(i):
            for t in range(NH2):
                nc.sync.dma_start(
                    out=qfT[t][:, i * 128:(i + 1) * 128],
                    in_=qf_b[i][:, t * 128:(t + 1) * 128], transpose=True)

        # ---------------- attention + output projection ----------------
        def scores_t(i, t, on_dve):
            """Diag-block scores for head pair t: two 64-contraction matmuls
            into per-hh [128,128] psum tiles (baseline-proven shapes)."""
            sts = []
            for hh in range(2):
                pq = psts([128, 128])
                nc.tensor.matmul(
                    pq,
                    kfT[t][hh * 64:hh * 64 + 64, i * 128:(i + 1) * 128],
                    qfT[t][hh * 64:hh * 64 + 64, i * 128:(i + 1) * 128],
                    start=True, stop=True)
                st = st_p.tile([128, 128], BF16, tag="st", name="st")
                if on_dve:
                    nc.vector.tensor_mul(st, pq, maskf4[:, 0:128])
                else:
                    nc.scalar.copy(st, pq)
                    nc.gpsimd.tensor_mul(st, st, maskf4[:, 0:128])
                sts.append(st)
            return sts

        def pa_t(i, t, sts, paqA, paqB):
            """Attention for head pair t into the packed psum (baseline
            layout: t 0-3 in paqA columns, t 4-5 in paqB)."""
            pa = (paqA[:, (t % 4) * 128:(t % 4) * 128 + 128] if t < 4
                  else paqB[:, (t - 4) * 128:(t - 4) * 128 + 128])
            for hh in range(2):
                h = 2 * t + hh
                dst = pa[hh * 64:hh * 64 + 64, :]
                if i > 0:
                    nc.tensor.matmul(
                        dst,
                        NP[i - 1][hh * 64:hh * 64 + 64, t * F:(t + 1) * F],
                        qfT[t][hh * 64:hh * 64 + 64, i * 128:(i + 1) * 128],
                        start=True, stop=False, skip_group_check=True)
                nc.tensor.matmul(
                    dst, v_p[i][:, h * F:(h + 1) * F], sts[hh],
                    start=(i == 0), stop=True, skip_group_check=True)

        def aT_evac_pair(i, t, paqA, paqB, on_dve):
            """Evacuate heads pairs t-1, t (t odd) like the baseline."""
            if t < 4:
                src = (paqA.rearrange("p (t l) -> p t l", l=128)
                       [:, t - 1:t + 1, :])
            else:
                src = paqB.rearrange("p (t l) -> p t l", l=128)
            dst = aTv[:, t - 1:t + 1, i * 128:(i + 1) * 128]
            if on_dve:
                nc.vector.tensor_copy(dst, src)
            else:
                nc.scalar.copy(dst, src)

        def outproj(i, tt, po1, po2):
            st0 = zb and tt == 0
            sp = tt == NH2 - 1
            nc.tensor.matmul(po1, aT_all[tt][:, i * 128:(i + 1) * 128],
                             wo[:, tt * 768:tt * 768 + 512],
                             start=st0, stop=sp, skip_group_check=True)
            nc.tensor.matmul(po2, aT_all[tt][:, i * 128:(i + 1) * 128],
                             wo[:, tt * 768 + 512:tt * 768 + 768],
                             start=st0, stop=sp, skip_group_check=True)

        def iblk(i, extra=()):
            """Attention + output projection for block i; `extra` stages are
            interleaved to fill engine slack."""
            extra = list(extra)
            po1 = pst([128, 512])
            po2 = pst([128, 256])
            if not zb:
                nc.tensor.matmul(po1, ones1, b_orow[0:1, 0:512],
                                 start=True, stop=False,
                                 skip_group_check=True)
                nc.tensor.matmul(po2, ones1, b_orow[0:1, 512:768],
                                 start=True, stop=False,
                                 skip_group_check=True)
            paqA = pst([128, 512])
            paqB = pst([128, 256])
            for t in range(NH2):
                sts = scores_t(i, t, t % 2 == 0)
                if t % 2 == 1 and extra:
                    extra.pop(0)()
                pa_t(i, t, sts, paqA, paqB)
                if t % 2 == 1:
                    aT_evac_pair(i, t, paqA, paqB, t == 3)
                    outproj(i, t - 1, po1, po2)
                    outproj(i, t, po1, po2)
            # output: evac + DMA (DVE/Act split + chunked DMAs to minimize
            # the kernel tail)
            osb = osb_p.tile([128, E], F32, tag="osb", name="osb")
            if i == LT - 1:
                nc.vector.tensor_copy(osb[:, 0:256], po1[:, 0:256])
                nc.scalar.copy(osb[:, 512:768], po2)
                nc.vector.tensor_copy(osb[:, 256:512], po1[:, 256:512])
                for c0 in (512, 0, 256):
                    nc.sync.dma_start(
                        out=out_d[i * 128:(i + 1) * 128, c0:c0 + 256],
                        in_=osb[:, c0:c0 + 256])
            else:
                nc.vector.tensor_copy(osb[:, 0:512], po1)
                nc.scalar.copy(osb[:, 512:768], po2)
                nc.sync.dma_start(
                    out=out_d[i * 128:(i + 1) * 128, 0:512],
                    in_=osb[:, 0:512])
                nc.sync.dma_start(
                    out=out_d[i * 128:(i + 1) * 128, 512:768],
                    in_=osb[:, 512:768])
            for fn in extra:
                fn()

        def qstage(j):
            sA, sB, _ = qkv_mm(0, j)
            qstage_exp(j, sA, sB)

        def bstage(j):
            k1stage(j)
            denstage(j)
            qtstage(j)

        # ================= emission =================
        def ckpt(n):
            if cut <= n or n == -1:
                # dummy out writes so the output is retrievable
                dummy = P.tile([128, E], F32, tag="dummy", name="dummy")
                nc.gpsimd.memset(dummy, 0.0)
                for ii in range(LT):
                    nc.sync.dma_start(
                        out=out_d[ii * 128:(ii + 1) * 128, :], in_=dummy)
                raise _Cut

        # Priority spine: k features -> q features -> K1/den/qfT chains,
        # with v projections and attention blocks filling in behind.
        ckpt(0)
        mark("kstage0")
        kstage_mm(0)
        mark("qstage0")
        qstage(0)
        ckpt(1)
        mark("kfac0")
        kstage_fac(0)
        ckpt(2)
        mark("bstage0")
        bstage(0)
        mark("kmm1")
        kstage_mm(1)
        kstage_fac(1)
        mark("qstage1")
        qstage(1)
        mark("bstage1")
        bstage(1)
        ckpt(3)
        mark("kmm23")
        kstage_mm(2)
        kstage_fac(2)
        kstage_mm(3)
        kstage_fac(3)
        ckpt(4)
        mark("vstage0")
        vstage(0)
        njstage(0)
        mark("qstage2")
        qstage(2)
        mark("vstage1")
        vstage(1)
        njstage(1)
        nc.gpsimd.tensor_add(NP[1], NP[0], pnb[1])
        mark("vstage23")
        vstage(2)
        njstage(2)
        nc.gpsimd.tensor_add(NP[2], NP[1], pnb[2])
        vstage(3)

        ckpt(5)
        mark("iblk0")
        iblk(0, extra=[lambda: (k1stage(2), denstage(2)),
                       lambda: qtstage(2)])
        mark("qstage3")
        qstage(3)
        mark("iblk1")
        iblk(1, extra=[lambda: (k1stage(3), denstage(3)),
                       lambda: qtstage(3)])
        ckpt(6)
        mark("iblk2")
        iblk(2)
        mark("iblk3")
        iblk(3)

      except _Cut:
        pass

    if fix_waits:
        # production wait-splitting: matmul waits migrate to ldweights, the
        # rest are split via InstEventSemaphore (walrus allows 1 wait/inst)
        bass_rust.move_matmul_waits_to_ldweights(nc.m)
        bass_rust.generate_event_semaphores(nc)
    return nc


_CACHE = {}


def _host_consts(wsumb):
    import ml_dtypes
    bf = ml_dtypes.bfloat16
    tri = np.triu(np.ones((128, 128), dtype=np.float32))
    masks = np.concatenate(
        [tri, np.tril(np.ones((128, 128), dtype=np.float32), -1),
         np.tile(tri, (1, 4))], axis=1).astype(bf)
    idb = np.eye(128, dtype=np.float32).astype(bf)
    return {"consts": np.concatenate([masks, wsumb, idb], axis=1)}


def _pair_pack(w, cols):
    """[768, cols] -> [128, PR*2*cols] fp8 e-pair/plane-major layout."""
    import ml_dtypes
    f8 = ml_dtypes.float8_e4m3
    return np.ascontiguousarray(
        w.reshape(PR, 2, 128, cols).transpose(2, 0, 1, 3)
        .reshape(128, PR * 2 * cols)).astype(f8)


def _in_maps(x, w_inp, b_inp, w_out, b_out, omega):
    import ml_dtypes
    bf = ml_dtypes.bfloat16
    f = lambda a: np.ascontiguousarray(np.asarray(a), dtype=np.float32)
    x, w_inp, b_inp = f(x), f(w_inp), f(b_inp)
    w_out, b_out, omega = f(w_out), f(b_out), f(omega)
    w = w_inp[0]  # [E, 3E]
    omt = (omega.T * (float(Dh) ** -0.25)).astype(np.float64)   # [d, f]
    # fold omega into the q/k projections: Ws[:, (qk,h,f)] per head
    ws = np.empty((E, 1536), np.float64)
    wqk_full = w[:, 0:1536].astype(np.float64)
    for qk in range(2):
        for h in range(H):
            c = qk * 768 + h * 64
            ws[:, c:c + 64] = wqk_full[:, c:c + 64] @ omt
    # k-side per-head column sums (diag), padded 12->16, bf16 et-major
    wsum_full = np.zeros((E, 16), np.float64)
    wsum_full[:, 0:12] = ws[:, 768:1536].reshape(E, 12, 64).sum(axis=2)
    wqk8 = _pair_pack((ws * W8SCALE).astype(np.float32), 1536)
    wsumb = np.ascontiguousarray(
        wsum_full.astype(np.float32).reshape(ET, 128, 16)
        .transpose(1, 0, 2).reshape(128, ET * 16)).astype(bf)
    consts = _host_consts(wsumb)
    wv8 = _pair_pack(w[:, 1536:2304] * W8SCALE, 768)
    # bf16 v weights (block 0), same x64 scale so v'=64v uniformly
    wvb = np.ascontiguousarray(
        (w[:, 1536:2304] * W8SCALE).reshape(ET, 128, 768)
        .transpose(1, 0, 2).reshape(128, ET * 768)).astype(bf)
    # wo/64 un-does the v'=64v scaling
    wo = np.ascontiguousarray(
        (w_out[0] * IS).reshape(ET, 128, 768).transpose(1, 0, 2)
        .reshape(128, ET * 768)).astype(bf)
    zb = bool(np.all(b_inp == 0.0) and np.all(b_out == 0.0))
    maps = []
    for c in range(B):
        xT = x[c].T
        x8 = _pair_pack(xT, L)
        xbn = np.ascontiguousarray(
            xT.reshape(ET, 128, L).transpose(1, 0, 2)
            .reshape(128, ET * L)).astype(bf)
        m = {"x8": x8, "xb": xbn, "wqk8": wqk8, "wv8": wv8, "wvb": wvb,
             "wo": wo}
        if not zb:
            bs = np.zeros((2, 1536 + 16), np.float32)
            for qk in range(2):
                bq = b_inp[qk * 768:(qk + 1) * 768].astype(np.float64)
                bsh = np.empty((768,), np.float64)
                for h in range(H):
                    bsh[h * 64:(h + 1) * 64] = bq[h * 64:(h + 1) * 64] @ omt
                # bias rows feed the x64-scaled psum: multiply by 64;
                # the pd psum is unscaled bf16: sums stay unscaled
                bs[qk, 0:768] = (bsh * W8SCALE).astype(np.float32)
                if qk == 1:
                    bs[1, 1536:1548] = (
                        bsh.reshape(12, 64).sum(axis=1)).astype(np.float32)
            m["bs_rows"] = bs
            m["ones1"] = np.ones((1, 128), np.float32)
            m["b_vv"] = np.ascontiguousarray(np.broadcast_to(
                b_inp[1536:2304] * W8SCALE, (128, E))).astype(np.float32)
            m["b_orow"] = np.ascontiguousarray(b_out).reshape(1, E)
        m.update(consts)
        maps.append(m)
    return maps


def kernel(x, w_inp, b_inp, w_out, b_out, omega):
    maps = _in_maps(x, w_inp, b_inp, w_out, b_out, omega)
    zb = "b_vv" not in maps[0]
    key = f"nc{int(zb)}"
    if key not in _CACHE:
        _CACHE[key] = build_nc(zb=zb)
    nc = _CACHE[key]
    res = bass_utils.run_bass_kernel_spmd(nc, maps, core_ids=list(range(B)))
    return np.stack([res.results[c]["out"] for c in range(B)])
